# revision 42
# baseline (speedup 1.0000x reference)
"""GCN (3-layer, JK-concat) Trainium2 kernel, 8-core SPMD.

Strategy (graph/data parallel, dst-sharded):
 - Nodes sorted by degree (desc), dealt round-robin to 8 cores -> each core
   owns 1250 dst nodes (padded to 1280 = 10 tiles of 128 partitions).
 - Symmetric norm folding: x is pre-scaled by dinv on host. When the BN/bias
   constant c == 0 (the shipped regime), the per-layer activation is kept in
   "scaled" form h~ = dinv*h = relu(dinv^2 * agg), which makes the next
   layer's table t'_{l+1} = h~ @ W'_{l+1} directly (no per-row scale), with a
   single 1/dinv row-scale applied once at the JK output.
 - Layer 1 needs NO collective: x is replicated input (xTf), so every core
   computes the ENTIRE t1' table locally (80 stripes of GEMM written to a
   local DRAM table). This removes AllGather #1 from the critical path.
 - Layers 2/3: t' tables are kept in fp8(e4m3), halving the AllGather
   bytes (~81us each in the CoreSim cost model vs ~147us for bf16). The
   collective's in/out APs are bitcast to bf16 containers so the NEFF sees
   an ordinary bf16 AllGather — declaring the collective tensors as fp8
   hard-crashes NRT exec on silicon (prior session), the bitcast does not.
 - Aggregation: per dst-tile dma_gather in balanced <=40-slot pieces
   (slot-major, degree-padded) -> identity-matmul scatter-accumulate into
   PSUM (fp32; fp8 identity/messages for layers 2/3) -> relu-scale (DVE)
   -> transpose (PE+ACT) for the next GEMM's lhsT. Self-loop messages are
   NOT gathered: each core keeps its own t' tiles in SBUF (own_t, one per
   layer with the layer's table dtype) and injects them with one extra
   accumulate-matmul per tile. The next layer's GEMM, bounce write, and
   the final JK linear are folded into the per-tile loop so they overlap
   the gather stream.
 - JK: out = (sum_l h~T_l.T @ lin_w_l) * (1/dinv) (+ lin_b), fp32 out.
"""
import sys
sys.path.insert(0, "/opt/trn_rl_repo")
import numpy as np
import ml_dtypes

import concourse.bass as bass
import concourse.bacc as bacc
import concourse.mybir as mybir
import concourse.tile as tile
from concourse.bass_utils import run_bass_kernel_spmd

N = 10000
E = 320000
D = 256
L = 3
BN_EPS = 1e-5
NCORES = 8
PPC = 1280           # padded nodes per core
NT = PPC // 128      # dst tiles per core
TBL = NCORES * PPC   # gather table rows
ZROW = 1279          # an always-zero table row (core 0 pad region)
BF16 = ml_dtypes.bfloat16

_TRACE = False       # set by test harness for profiling runs


# ----------------------------------------------------------------- host prep
def _preprocess(x, edge_index, Ws, bs, bn_scale, bn_bias, bn_mean, bn_var,
                lin_w, lin_b):
    src = np.asarray(edge_index[0], np.int64)
    dst = np.asarray(edge_index[1], np.int64)
    loops = np.arange(N, dtype=np.int64)

    # degree INCLUDES the self-loop (reference semantics) ...
    deg = (np.bincount(dst, minlength=N) + 1).astype(np.float64)
    dinv = np.where(deg > 0, 1.0 / np.sqrt(deg), 0.0).astype(np.float32)

    # deg-sorted round-robin node placement
    order = np.argsort(-deg, kind="stable")          # node of rank r
    core_of_rank = np.arange(N) % NCORES
    local_of_rank = np.arange(N) // NCORES
    core = np.empty(N, np.int64); local = np.empty(N, np.int64)
    core[order] = core_of_rank
    local[order] = local_of_rank
    trow = core * PPC + local                        # table row per node

    # ... but the gather schedule EXCLUDES self-loops: the own-t' term is
    # injected on-device from SBUF, saving one slot in every bucket.
    e_core = core[dst]
    e_tile = local[dst] // 128
    e_part = local[dst] % 128
    e_src_row = trow[src]

    # slot index of each edge within its (core, tile, part) bucket
    key = (e_core * NT + e_tile) * 128 + e_part
    order_e = np.argsort(key, kind="stable")
    key_s = key[order_e]
    starts = np.searchsorted(key_s, np.arange(NCORES * NT * 128))
    counts = np.diff(np.append(starts, len(key_s)))
    slot_in_bucket = np.arange(len(key_s)) - starts[key_s]

    counts3 = counts.reshape(NCORES, NT, 128)
    S = counts3.max(axis=(0, 2))                     # common slots per tile
    S = np.maximum(S, 1).astype(np.int64)

    # index arrays: per core, per tile: [S_b * 128] slot-major
    idx_lists = np.full((NCORES, int(S.sum()) * 128), ZROW, np.int32)
    tile_off = np.concatenate([[0], np.cumsum(S)])   # in slots
    ec = e_core[order_e]; et = e_tile[order_e]; ep = e_part[order_e]
    sl = slot_in_bucket
    flat_pos = (tile_off[et] + sl) * 128 + ep
    idx_lists[ec, flat_pos] = e_src_row[order_e]

    # wrap for dma_gather: position i -> partition i%16, col i//16; replicate x8
    tot = idx_lists.shape[1]
    assert tot % 16 == 0
    wrapped = idx_lists.reshape(NCORES, tot // 16, 16).transpose(0, 2, 1)
    idx_in = np.ascontiguousarray(
        np.tile(wrapped, (1, 8, 1))).astype(np.int16)  # [NCORES,128,tot//16]

    # BN folding
    rs = 1.0 / np.sqrt(np.asarray(bn_var, np.float64) + BN_EPS)
    colscale = rs * np.asarray(bn_scale, np.float64)           # [L,D]
    Wp = np.asarray(Ws, np.float64) * colscale[:, None, :]     # [L,D,D]
    c = ((np.asarray(bs, np.float64) - np.asarray(bn_mean, np.float64))
         * colscale + np.asarray(bn_bias, np.float64))         # [L,D]
    fold = bool(np.all(np.abs(c) < 1e-12))

    # per-core dinv-prescaled x^T
    xs = np.asarray(x, np.float32) * dinv[:, None]
    xT_pc = np.zeros((NCORES, D, PPC), np.float32)
    for cc in range(NCORES):
        nodes = order[cc::NCORES]
        xT_pc[cc, :, :len(nodes)] = xs[nodes].T
    # full table-ordered x^T (same on every core): lets each core compute the
    # ENTIRE layer-1 t' table locally, replacing the first AllGather.
    xT_full = np.zeros((D, TBL), np.float32)
    for cc in range(NCORES):
        nodes = order[cc::NCORES]
        xT_full[:, cc * PPC:cc * PPC + len(nodes)] = xs[nodes].T

    # per-core dst-tile dinv vectors: dinv^2 (fold) or dinv (generic), plus
    # 1/dinv for the JK output (fold only; 0 at pads).
    dinv_t = np.zeros((NCORES, 128, NT), np.float32)
    recip_t = np.zeros((NCORES, 128, NT), np.float32)
    for cc in range(NCORES):
        nodes = order[cc::NCORES]                    # nodes of core cc by local
        dv = np.zeros(PPC, np.float32)
        dv[:len(nodes)] = dinv[nodes]
        dvt = dv.reshape(NT, 128).T
        dinv_t[cc] = dvt * dvt if fold else dvt
        with np.errstate(divide="ignore"):
            recip_t[cc] = np.where(dvt > 0, 1.0 / dvt, 0.0)
    return dict(
        S=S, idx_in=idx_in, xT_pc=xT_pc.astype(BF16),
        xT_full=np.ascontiguousarray(xT_full.astype(BF16)), dinv_t=dinv_t,
        recip_t=recip_t, fold=fold,
        Wp=Wp.astype(np.float32), c=c.astype(np.float32),
        lin_w=np.asarray(lin_w, np.float32), lin_b=np.asarray(lin_b, np.float32),
        order=order,
    )


# -------------------------------------------------------------- device build
def _build(S, Wp, c, lin_w, lin_b, fold):
    S = [int(s) for s in S]
    tot = sum(S) * 128
    b_zero = bool(np.all(np.abs(lin_b) < 1e-12))

    nc = bacc.Bacc("TRN2", target_bir_lowering=False, debug=False,
                   enable_asserts=True, num_devices=NCORES)
    xT_d = nc.dram_tensor("xT", [D, PPC], mybir.dt.bfloat16,
                          kind="ExternalInput")
    xTf_d = nc.dram_tensor("xTf", [D, TBL], mybir.dt.bfloat16,
                           kind="ExternalInput")
    idx_d = nc.dram_tensor("idx", [128, tot // 16], mybir.dt.int16,
                           kind="ExternalInput")
    dinv_d = nc.dram_tensor("dinv", [128, NT], mybir.dt.float32,
                            kind="ExternalInput")
    recip_d = nc.dram_tensor("recip", [128, NT], mybir.dt.float32,
                             kind="ExternalInput")
    out_d = nc.dram_tensor("out", [PPC, D], mybir.dt.float32,
                           kind="ExternalOutput")

    ident_d = nc.inline_tensor(np.eye(128, dtype=BF16), name="ident")
    ident8_d = nc.inline_tensor(
        np.eye(128).astype(ml_dtypes.float8_e4m3), name="ident8")
    Wp_d = nc.inline_tensor(Wp.astype(BF16), name="Wp")            # [L,D,D]
    linw_d = nc.inline_tensor(lin_w.astype(BF16), name="linw")     # [768,D]
    if not fold:
        c_d = nc.inline_tensor(
            np.broadcast_to(c[:, None, :], (L, 128, D)).copy(), name="cvec")
    if not b_zero:
        linb_d = nc.inline_tensor(
            np.broadcast_to(lin_b[None, :], (128, D)).copy(), name="linb")

    with tile.TileContext(nc) as tc:
        with (
            tc.tile_pool(name="const", bufs=1) as constp,
            tc.tile_pool(name="hT", bufs=1) as hTp,
            tc.tile_pool(name="msg", bufs=2) as msgp,
            tc.tile_pool(name="work", bufs=3) as workp,
            tc.tile_pool(name="dram", bufs=2, space="DRAM") as dramp,
            tc.tile_pool(name="psA", bufs=2, space="PSUM") as psA,
            tc.tile_pool(name="psB", bufs=2, space="PSUM") as psB,
            tc.tile_pool(name="psT", bufs=2, space="PSUM") as psT,
        ):
            # ---- critical-path constants first: W (for the L1 GEMM), x^T.
            W_sb = constp.tile([128, L * 2, D], mybir.dt.bfloat16, tag="W")
            nc.sync.dma_start(
                W_sb[:], Wp_d.ap().rearrange("l (h p) d -> p (l h) d", p=128))
            xT_sb = constp.tile([128, 2, PPC], mybir.dt.bfloat16, tag="xTpc")
            for h in range(2):
                nc.sync.dma_start(xT_sb[:, h, :],
                                  xT_d[128 * h:128 * (h + 1), :])
            # full table-ordered x^T: every core computes the ENTIRE layer-1
            # t' table locally (x is replicated input), replacing AG #1.
            # Two column-half tiles so the GEMM stream starts after the
            # first half's load instead of the full 5.2MB.
            HTB = TBL // 2
            xTfs = []
            for half in range(2):
                t = constp.tile([128, 2, HTB], mybir.dt.bfloat16,
                                tag=f"xTf{half}")
                for h in range(2):
                    nc.sync.dma_start(
                        t[:, h, :],
                        xTf_d[128 * h:128 * (h + 1),
                              half * HTB:(half + 1) * HTB])
                xTfs.append(t)

            # own-t' SBUF stash, one per layer (L1 bf16; L2/L3 fp8 to halve
            # the AllGather bytes). Feeds the self-loop injection.
            FP8 = mybir.dt.float8e4
            own_t = [hTp.tile([128, NT, D],
                              mybir.dt.bfloat16 if i == 0 else FP8,
                              tag=f"own_{i}", name=f"own_t{i}")
                     for i in range(L)]

            # ---- layer-1: replicated GEMM over all 80 table stripes,
            # written straight to the local DRAM table (no collective).
            # This gates the first gather, so it runs FIRST on the PE.
            table1 = dramp.tile([TBL, D], mybir.dt.bfloat16, tag="table1")
            G1 = 4                      # stripes per staged table write
            for g in range(TBL // 128 // G1):
                stage = workp.tile([128, G1, D], mybir.dt.bfloat16,
                                   tag="l1stage")
                for j in range(G1):
                    s = g * G1 + j
                    src = xTfs[s // (HTB // 128)]
                    so = s % (HTB // 128)
                    tp = psB.tile([128, D], mybir.dt.float32, tag="gemm")
                    for h in range(2):
                        nc.tensor.matmul(
                            tp[:], src[:, h, 128 * so:128 * (so + 1)],
                            W_sb[:, h, :], start=(h == 0), stop=(h == 1))
                    if j % 2 == 0:
                        nc.scalar.activation(
                            stage[:, j, :], tp[:],
                            mybir.ActivationFunctionType.Copy)
                    else:
                        nc.vector.tensor_copy(stage[:, j, :], tp[:])
                nc.sync.dma_start(
                    table1[128 * G1 * g:128 * G1 * (g + 1), :]
                    .rearrange("(k p) d -> p k d", p=128),
                    stage[:])
            # own tiles for the self-loop injection: not needed until the
            # tile loop, so this GEMM runs AFTER the table-write stream.
            for b in range(NT):
                tp = psB.tile([128, D], mybir.dt.float32, tag="gemm")
                for h in range(2):
                    nc.tensor.matmul(
                        tp[:], xT_sb[:, h, 128 * b:128 * (b + 1)],
                        W_sb[:, h, :], start=(h == 0), stop=(h == 1))
                if b % 2 == 0:
                    nc.scalar.activation(
                        own_t[0][:, b, :], tp[:],
                        mybir.ActivationFunctionType.Copy)
                else:
                    nc.vector.tensor_copy(own_t[0][:, b, :], tp[:])

            # ---- remaining constants (overlap the collective)
            ident = constp.tile([128, 128], mybir.dt.bfloat16, tag="ident")
            nc.sync.dma_start(ident[:], ident_d[:])
            ident8 = constp.tile([128, 128], FP8, tag="ident8")
            nc.sync.dma_start(ident8[:], ident8_d[:])


            idx_sb = constp.tile([128, tot // 16], mybir.dt.int16, tag="idx")
            nc.sync.dma_start(idx_sb[:], idx_d[:])
            dinv_sb = constp.tile([128, NT], mybir.dt.float32, tag="dinv")
            nc.sync.dma_start(dinv_sb[:], dinv_d[:])
            if fold:
                recip_sb = constp.tile([128, NT], mybir.dt.float32, tag="recip")
                nc.sync.dma_start(recip_sb[:], recip_d[:])
            linw_sb = constp.tile([128, L * 2, D], mybir.dt.bfloat16,
                                  tag="linw")
            nc.sync.dma_start(
                linw_sb[:], linw_d.ap().rearrange("(k p) d -> p k d", p=128))
            if not fold:
                c_sb = constp.tile([128, L, D], mybir.dt.float32, tag="cvec")
                nc.sync.dma_start(c_sb[:], c_d.ap().rearrange("l p d -> p l d"))
            if not b_zero:
                linb_sb = constp.tile([128, D], mybir.dt.float32, tag="linb")
                nc.sync.dma_start(linb_sb[:], linb_d.ap())

            hT = None
            hT_layers = []
            bounce = None
            table = table1
            for l in range(L):
                tdt = mybir.dt.bfloat16 if l == 0 else FP8
                idt = ident if l == 0 else ident8
                if l > 0:
                    # ---- AllGather the fp8 t' table (bounce written during
                    # the previous layer's tile loop). The collective's APs
                    # are bitcast to bf16 so the NEFF sees a plain bf16
                    # AllGather of half the bytes.
                    table = dramp.tile([TBL, D], FP8,
                                       tag="table", addr_space="Shared")
                    nc.gpsimd.collective_compute(
                        "AllGather", mybir.AluOpType.bypass,
                        replica_groups=[list(range(NCORES))],
                        ins=[bounce.opt().bitcast(mybir.dt.bfloat16)],
                        outs=[table.opt().bitcast(mybir.dt.bfloat16)])
                if l < L - 1:
                    bounce = dramp.tile([PPC, D], FP8, tag="bounce")
                own_cur = own_t[l]
                own_nxt = own_t[l + 1] if l < L - 1 else None

                # ---- gather + scatter + relu (+ next-layer GEMM / JK) per tile
                new_hT = {}
                for h in range(2):
                    nht = hTp.tile([128, PPC], mybir.dt.bfloat16,
                                   tag=f"hT_{l}_{h}")
                    new_hT[h] = nht
                off = 0
                for b in range(NT):
                    sb_ = S[b]
                    agg = psA.tile([128, D], mybir.dt.float32, tag="agg")
                    # gather in balanced pieces of <=40 slots: caps the msg
                    # pool slot size without tiny tail pieces, and lets the
                    # scatter matmuls of piece k overlap gather k+1. The
                    # last tile before an AllGather is force-split so its
                    # scatter overlaps the second piece, shrinking the
                    # pre-AG tail.
                    np_ = -(-sb_ // 40)
                    if b == NT - 1:
                        np_ = max(np_, 2)
                    cuts = [sb_ * i // np_ for i in range(np_ + 1)]
                    pieces = list(zip(cuts[:-1], cuts[1:]))
                    for pi, (p0, p1) in enumerate(pieces):
                        msg = msgp.tile([128, p1 - p0, D], tdt,
                                        tag=f"msg{min(l, 1)}")
                        nc.gpsimd.dma_gather(
                            msg[:], table[:],
                            idx_sb[:, (off + p0 * 128) // 16:
                                   (off + p1 * 128) // 16],
                            num_idxs=(p1 - p0) * 128,
                            num_idxs_reg=(p1 - p0) * 128, elem_size=D,
                            single_packet=False)
                        for s in range(p1 - p0):
                            nc.tensor.matmul(agg[:], idt[:], msg[:, s, :],
                                             start=(pi == 0 and s == 0),
                                             stop=False)
                    off += sb_ * 128
                    # self-loop term from the SBUF stash (not gathered)
                    nc.tensor.matmul(agg[:], idt[:], own_cur[:, b, :],
                                     start=False, stop=True)
                    # h~ = relu(dinv^2 * agg)  (fold)  /  relu(dinv*agg + c)
                    hb = workp.tile([128, D], mybir.dt.bfloat16, tag="hb")
                    if fold:
                        nc.vector.tensor_scalar(
                            hb[:], agg[:], dinv_sb[:, b:b + 1], 0.0,
                            mybir.AluOpType.mult, mybir.AluOpType.max)
                    else:
                        tmp = workp.tile([128, D], mybir.dt.float32, tag="tmp")
                        nc.vector.tensor_scalar(
                            tmp[:], agg[:], dinv_sb[:, b:b + 1], None,
                            mybir.AluOpType.mult)
                        nc.vector.tensor_tensor(
                            tmp[:], tmp[:], c_sb[:, l, :], mybir.AluOpType.add)
                        nc.vector.tensor_scalar(
                            hb[:], tmp[:], 0.0, None, mybir.AluOpType.max)
                    # transpose h tile -> hT
                    for h in range(2):
                        tps = psT.tile([128, 128], mybir.dt.float32, tag="tr")
                        nc.tensor.matmul(tps[:], hb[:, 128 * h:128 * (h + 1)],
                                         ident[:], start=True, stop=True)
                        nc.scalar.activation(
                            new_hT[h][:, 128 * b:128 * (b + 1)], tps[:],
                            mybir.ActivationFunctionType.Copy)

                    if l < L - 1:
                        # next layer's local GEMM tile -> own stash -> bounce
                        tp = psB.tile([128, D], mybir.dt.float32, tag="gemm")
                        for h in range(2):
                            nc.tensor.matmul(
                                tp[:], new_hT[h][:, 128 * b:128 * (b + 1)],
                                W_sb[:, 2 * (l + 1) + h, :],
                                start=(h == 0), stop=(h == 1))
                        if fold:
                            nc.scalar.activation(
                                own_nxt[:, b, :], tp[:],
                                mybir.ActivationFunctionType.Copy)
                        else:
                            nc.vector.tensor_scalar(
                                own_nxt[:, b, :], tp[:], dinv_sb[:, b:b + 1],
                                None, mybir.AluOpType.mult)
                        nc.sync.dma_start(bounce[128 * b:128 * (b + 1), :],
                                          own_nxt[:, b, :])
                    else:
                        # JK final linear for this dst tile
                        op = psB.tile([128, D], mybir.dt.float32, tag="gemm")
                        k = 0
                        for l2 in range(L):
                            hTl = new_hT if l2 == l else hT_layers[l2]
                            for h in range(2):
                                nc.tensor.matmul(
                                    op[:], hTl[h][:, 128 * b:128 * (b + 1)],
                                    linw_sb[:, 2 * l2 + h, :],
                                    start=(k == 0), stop=(k == 5))
                                k += 1
                        ob = workp.tile([128, D], mybir.dt.float32, tag="ob")
                        if fold:
                            nc.vector.tensor_scalar(
                                ob[:], op[:], recip_sb[:, b:b + 1],
                                None, mybir.AluOpType.mult)
                        else:
                            nc.vector.tensor_copy(ob[:], op[:])
                        if not b_zero:
                            nc.vector.tensor_tensor(
                                ob[:], ob[:], linb_sb[:], mybir.AluOpType.add)
                        nc.sync.dma_start(out_d[128 * b:128 * (b + 1), :],
                                          ob[:])
                hT = new_hT
                hT_layers.append(new_hT)
    nc.compile()
    return nc


# ------------------------------------------------------------------- runner
def _in_maps(pre):
    return [
        {"xT": np.ascontiguousarray(pre["xT_pc"][cc]),
         "xTf": pre["xT_full"],
         "idx": np.ascontiguousarray(pre["idx_in"][cc]),
         "dinv": np.ascontiguousarray(pre["dinv_t"][cc]),
         "recip": np.ascontiguousarray(pre["recip_t"][cc])}
        for cc in range(NCORES)
    ]


def _unshard(pre, results):
    order = pre["order"]
    out = np.empty((N, D), np.float32)
    for cc in range(NCORES):
        nodes = order[cc::NCORES]
        out[nodes] = results[cc]["out"][:len(nodes)]
    return out


def kernel(x, edge_index, Ws, bs, bn_scale, bn_bias, bn_mean, bn_var,
           lin_w, lin_b):
    pre = _preprocess(x, edge_index, Ws, bs, bn_scale, bn_bias, bn_mean,
                      bn_var, lin_w, lin_b)
    nc = _build(pre["S"], pre["Wp"], pre["c"], pre["lin_w"], pre["lin_b"],
                pre["fold"])
    kw = {}
    if _TRACE:
        kw = dict(trace=True)
    res = run_bass_kernel_spmd(nc, _in_maps(pre), core_ids=list(range(NCORES)),
                               **kw)
    kernel.last_results = res
    return _unshard(pre, [res.results[cc] for cc in range(NCORES)])



# revision 43
# speedup vs baseline: 1.0009x; 1.0009x over previous
"""GCN (3-layer, JK-concat) Trainium2 kernel, 8-core SPMD.

Strategy (graph/data parallel, dst-sharded):
 - Nodes sorted by degree (desc), dealt round-robin to 8 cores -> each core
   owns 1250 dst nodes (padded to 1280 = 10 tiles of 128 partitions).
 - Symmetric norm folding: x is pre-scaled by dinv on host. When the BN/bias
   constant c == 0 (the shipped regime), the per-layer activation is kept in
   "scaled" form h~ = dinv*h = relu(dinv^2 * agg), which makes the next
   layer's table t'_{l+1} = h~ @ W'_{l+1} directly (no per-row scale), with a
   single 1/dinv row-scale applied once at the JK output.
 - Layer 1 needs NO collective: x is replicated input (xTf), so every core
   computes the ENTIRE t1' table locally (80 stripes of GEMM written to a
   local DRAM table). This removes AllGather #1 from the critical path.
 - Layers 2/3: t' tables are kept in fp8(e4m3), halving the AllGather
   bytes (~81us each in the CoreSim cost model vs ~147us for bf16). The
   collective's in/out APs are bitcast to bf16 containers so the NEFF sees
   an ordinary bf16 AllGather — declaring the collective tensors as fp8
   hard-crashes NRT exec on silicon (prior session), the bitcast does not.
 - Aggregation: per dst-tile dma_gather in balanced <=40-slot pieces
   (slot-major, degree-padded) -> identity-matmul scatter-accumulate into
   PSUM (fp32; fp8 identity/messages for layers 2/3) -> relu-scale (DVE)
   -> transpose (PE+ACT) for the next GEMM's lhsT. Self-loop messages are
   NOT gathered: each core keeps its own t' tiles in SBUF (own_t, one per
   layer with the layer's table dtype) and injects them with one extra
   accumulate-matmul per tile. The next layer's GEMM, bounce write, and
   the final JK linear are folded into the per-tile loop so they overlap
   the gather stream.
 - JK: out = (sum_l h~T_l.T @ lin_w_l) * (1/dinv) (+ lin_b), fp32 out.
"""
import sys
sys.path.insert(0, "/opt/trn_rl_repo")
import numpy as np
import ml_dtypes

import concourse.bass as bass
import concourse.bacc as bacc
import concourse.mybir as mybir
import concourse.tile as tile
from concourse.bass_utils import run_bass_kernel_spmd

N = 10000
E = 320000
D = 256
L = 3
BN_EPS = 1e-5
NCORES = 8
PPC = 1280           # padded nodes per core
NT = PPC // 128      # dst tiles per core
TBL = NCORES * PPC   # gather table rows
ZROW = 1279          # an always-zero table row (core 0 pad region)
BF16 = ml_dtypes.bfloat16

_TRACE = False       # set by test harness for profiling runs


# ----------------------------------------------------------------- host prep
def _preprocess(x, edge_index, Ws, bs, bn_scale, bn_bias, bn_mean, bn_var,
                lin_w, lin_b):
    src = np.asarray(edge_index[0], np.int64)
    dst = np.asarray(edge_index[1], np.int64)
    loops = np.arange(N, dtype=np.int64)

    # degree INCLUDES the self-loop (reference semantics) ...
    deg = (np.bincount(dst, minlength=N) + 1).astype(np.float64)
    dinv = np.where(deg > 0, 1.0 / np.sqrt(deg), 0.0).astype(np.float32)

    # deg-sorted round-robin node placement
    order = np.argsort(-deg, kind="stable")          # node of rank r
    core_of_rank = np.arange(N) % NCORES
    local_of_rank = np.arange(N) // NCORES
    core = np.empty(N, np.int64); local = np.empty(N, np.int64)
    core[order] = core_of_rank
    local[order] = local_of_rank
    trow = core * PPC + local                        # table row per node

    # ... but the gather schedule EXCLUDES self-loops: the own-t' term is
    # injected on-device from SBUF, saving one slot in every bucket.
    e_core = core[dst]
    e_tile = local[dst] // 128
    e_part = local[dst] % 128
    e_src_row = trow[src]

    # slot index of each edge within its (core, tile, part) bucket
    key = (e_core * NT + e_tile) * 128 + e_part
    order_e = np.argsort(key, kind="stable")
    key_s = key[order_e]
    starts = np.searchsorted(key_s, np.arange(NCORES * NT * 128))
    counts = np.diff(np.append(starts, len(key_s)))
    slot_in_bucket = np.arange(len(key_s)) - starts[key_s]

    counts3 = counts.reshape(NCORES, NT, 128)
    S = counts3.max(axis=(0, 2))                     # common slots per tile
    S = np.maximum(S, 1).astype(np.int64)

    # index arrays: per core, per tile: [S_b * 128] slot-major
    idx_lists = np.full((NCORES, int(S.sum()) * 128), ZROW, np.int32)
    tile_off = np.concatenate([[0], np.cumsum(S)])   # in slots
    ec = e_core[order_e]; et = e_tile[order_e]; ep = e_part[order_e]
    sl = slot_in_bucket
    flat_pos = (tile_off[et] + sl) * 128 + ep
    idx_lists[ec, flat_pos] = e_src_row[order_e]

    # wrap for dma_gather: position i -> partition i%16, col i//16; replicate x8
    tot = idx_lists.shape[1]
    assert tot % 16 == 0
    wrapped = idx_lists.reshape(NCORES, tot // 16, 16).transpose(0, 2, 1)
    idx_in = np.ascontiguousarray(
        np.tile(wrapped, (1, 8, 1))).astype(np.int16)  # [NCORES,128,tot//16]

    # BN folding
    rs = 1.0 / np.sqrt(np.asarray(bn_var, np.float64) + BN_EPS)
    colscale = rs * np.asarray(bn_scale, np.float64)           # [L,D]
    Wp = np.asarray(Ws, np.float64) * colscale[:, None, :]     # [L,D,D]
    c = ((np.asarray(bs, np.float64) - np.asarray(bn_mean, np.float64))
         * colscale + np.asarray(bn_bias, np.float64))         # [L,D]
    fold = bool(np.all(np.abs(c) < 1e-12))

    # per-core dinv-prescaled x^T
    xs = np.asarray(x, np.float32) * dinv[:, None]
    xT_pc = np.zeros((NCORES, D, PPC), np.float32)
    for cc in range(NCORES):
        nodes = order[cc::NCORES]
        xT_pc[cc, :, :len(nodes)] = xs[nodes].T
    # full table-ordered x^T (same on every core): lets each core compute the
    # ENTIRE layer-1 t' table locally, replacing the first AllGather.
    xT_full = np.zeros((D, TBL), np.float32)
    for cc in range(NCORES):
        nodes = order[cc::NCORES]
        xT_full[:, cc * PPC:cc * PPC + len(nodes)] = xs[nodes].T

    # per-core dst-tile dinv vectors: dinv^2 (fold) or dinv (generic), plus
    # 1/dinv for the JK output (fold only; 0 at pads).
    dinv_t = np.zeros((NCORES, 128, NT), np.float32)
    recip_t = np.zeros((NCORES, 128, NT), np.float32)
    for cc in range(NCORES):
        nodes = order[cc::NCORES]                    # nodes of core cc by local
        dv = np.zeros(PPC, np.float32)
        dv[:len(nodes)] = dinv[nodes]
        dvt = dv.reshape(NT, 128).T
        dinv_t[cc] = dvt * dvt if fold else dvt
        with np.errstate(divide="ignore"):
            recip_t[cc] = np.where(dvt > 0, 1.0 / dvt, 0.0)
    return dict(
        S=S, idx_in=idx_in, xT_pc=xT_pc.astype(BF16),
        xT_full=np.ascontiguousarray(xT_full.astype(BF16)), dinv_t=dinv_t,
        recip_t=recip_t, fold=fold,
        Wp=Wp.astype(np.float32), c=c.astype(np.float32),
        lin_w=np.asarray(lin_w, np.float32), lin_b=np.asarray(lin_b, np.float32),
        order=order,
    )


# -------------------------------------------------------------- device build
def _build(S, Wp, c, lin_w, lin_b, fold):
    S = [int(s) for s in S]
    tot = sum(S) * 128
    b_zero = bool(np.all(np.abs(lin_b) < 1e-12))

    nc = bacc.Bacc("TRN2", target_bir_lowering=False, debug=False,
                   enable_asserts=True, num_devices=NCORES)
    xT_d = nc.dram_tensor("xT", [D, PPC], mybir.dt.bfloat16,
                          kind="ExternalInput")
    xTf_d = nc.dram_tensor("xTf", [D, TBL], mybir.dt.bfloat16,
                           kind="ExternalInput")
    idx_d = nc.dram_tensor("idx", [128, tot // 16], mybir.dt.int16,
                           kind="ExternalInput")
    dinv_d = nc.dram_tensor("dinv", [128, NT], mybir.dt.float32,
                            kind="ExternalInput")
    recip_d = nc.dram_tensor("recip", [128, NT], mybir.dt.float32,
                             kind="ExternalInput")
    out_d = nc.dram_tensor("out", [PPC, D], mybir.dt.float32,
                           kind="ExternalOutput")

    ident_d = nc.inline_tensor(np.eye(128, dtype=BF16), name="ident")
    ident8_d = nc.inline_tensor(
        np.eye(128).astype(ml_dtypes.float8_e4m3), name="ident8")
    Wp_d = nc.inline_tensor(Wp.astype(BF16), name="Wp")            # [L,D,D]
    linw_d = nc.inline_tensor(lin_w.astype(BF16), name="linw")     # [768,D]
    if not fold:
        c_d = nc.inline_tensor(
            np.broadcast_to(c[:, None, :], (L, 128, D)).copy(), name="cvec")
    if not b_zero:
        linb_d = nc.inline_tensor(
            np.broadcast_to(lin_b[None, :], (128, D)).copy(), name="linb")

    with tile.TileContext(nc) as tc:
        with (
            tc.tile_pool(name="const", bufs=1) as constp,
            tc.tile_pool(name="hT", bufs=1) as hTp,
            tc.tile_pool(name="msg", bufs=2) as msgp,
            tc.tile_pool(name="work", bufs=3) as workp,
            tc.tile_pool(name="dram", bufs=2, space="DRAM") as dramp,
            tc.tile_pool(name="psA", bufs=2, space="PSUM") as psA,
            tc.tile_pool(name="psB", bufs=2, space="PSUM") as psB,
            tc.tile_pool(name="psT", bufs=2, space="PSUM") as psT,
        ):
            # ---- critical-path constants first: W (for the L1 GEMM), x^T.
            W_sb = constp.tile([128, L * 2, D], mybir.dt.bfloat16, tag="W")
            nc.sync.dma_start(
                W_sb[:], Wp_d.ap().rearrange("l (h p) d -> p (l h) d", p=128))
            xT_sb = constp.tile([128, 2, PPC], mybir.dt.bfloat16, tag="xTpc")
            for h in range(2):
                nc.sync.dma_start(xT_sb[:, h, :],
                                  xT_d[128 * h:128 * (h + 1), :])
            # full table-ordered x^T: every core computes the ENTIRE layer-1
            # t' table locally (x is replicated input), replacing AG #1.
            # Two column-half tiles so the GEMM stream starts after the
            # first half's load instead of the full 5.2MB.
            HTB = TBL // 2
            xTfs = []
            for half in range(2):
                t = constp.tile([128, 2, HTB], mybir.dt.bfloat16,
                                tag=f"xTf{half}")
                for h in range(2):
                    nc.sync.dma_start(
                        t[:, h, :],
                        xTf_d[128 * h:128 * (h + 1),
                              half * HTB:(half + 1) * HTB])
                xTfs.append(t)

            # own-t' SBUF stash, one per layer (L1 bf16; L2/L3 fp8 to halve
            # the AllGather bytes). Feeds the self-loop injection.
            FP8 = mybir.dt.float8e4
            own_t = [hTp.tile([128, NT, D],
                              mybir.dt.bfloat16 if i == 0 else FP8,
                              tag=f"own_{i}", name=f"own_t{i}")
                     for i in range(L)]

            # ---- layer-1: replicated GEMM over all 80 table stripes,
            # written straight to the local DRAM table (no collective).
            # Own tiles additionally land in own_t[0] for self-injection.
            for b in range(NT):
                tp = psB.tile([128, D], mybir.dt.float32, tag="gemm")
                for h in range(2):
                    nc.tensor.matmul(
                        tp[:], xT_sb[:, h, 128 * b:128 * (b + 1)],
                        W_sb[:, h, :], start=(h == 0), stop=(h == 1))
                if b % 2 == 0:
                    nc.scalar.activation(
                        own_t[0][:, b, :], tp[:],
                        mybir.ActivationFunctionType.Copy)
                else:
                    nc.vector.tensor_copy(own_t[0][:, b, :], tp[:])
            table1 = dramp.tile([TBL, D], mybir.dt.bfloat16, tag="table1")
            G1 = 4                      # stripes per staged table write
            for g in range(TBL // 128 // G1):
                stage = workp.tile([128, G1, D], mybir.dt.bfloat16,
                                   tag="l1stage")
                for j in range(G1):
                    s = g * G1 + j
                    src = xTfs[s // (HTB // 128)]
                    so = s % (HTB // 128)
                    tp = psB.tile([128, D], mybir.dt.float32, tag="gemm")
                    for h in range(2):
                        nc.tensor.matmul(
                            tp[:], src[:, h, 128 * so:128 * (so + 1)],
                            W_sb[:, h, :], start=(h == 0), stop=(h == 1))
                    if j % 2 == 0:
                        nc.scalar.activation(
                            stage[:, j, :], tp[:],
                            mybir.ActivationFunctionType.Copy)
                    else:
                        nc.vector.tensor_copy(stage[:, j, :], tp[:])
                nc.sync.dma_start(
                    table1[128 * G1 * g:128 * G1 * (g + 1), :]
                    .rearrange("(k p) d -> p k d", p=128),
                    stage[:])

            # ---- remaining constants (overlap the collective)
            ident = constp.tile([128, 128], mybir.dt.bfloat16, tag="ident")
            nc.sync.dma_start(ident[:], ident_d[:])
            ident8 = constp.tile([128, 128], FP8, tag="ident8")
            nc.sync.dma_start(ident8[:], ident8_d[:])


            idx_sb = constp.tile([128, tot // 16], mybir.dt.int16, tag="idx")
            nc.sync.dma_start(idx_sb[:], idx_d[:])
            dinv_sb = constp.tile([128, NT], mybir.dt.float32, tag="dinv")
            nc.sync.dma_start(dinv_sb[:], dinv_d[:])
            if fold:
                recip_sb = constp.tile([128, NT], mybir.dt.float32, tag="recip")
                nc.sync.dma_start(recip_sb[:], recip_d[:])
            linw_sb = constp.tile([128, L * 2, D], mybir.dt.bfloat16,
                                  tag="linw")
            nc.sync.dma_start(
                linw_sb[:], linw_d.ap().rearrange("(k p) d -> p k d", p=128))
            if not fold:
                c_sb = constp.tile([128, L, D], mybir.dt.float32, tag="cvec")
                nc.sync.dma_start(c_sb[:], c_d.ap().rearrange("l p d -> p l d"))
            if not b_zero:
                linb_sb = constp.tile([128, D], mybir.dt.float32, tag="linb")
                nc.sync.dma_start(linb_sb[:], linb_d.ap())

            hT = None
            hT_layers = []
            bounce = None
            table = table1
            for l in range(L):
                tdt = mybir.dt.bfloat16 if l == 0 else FP8
                idt = ident if l == 0 else ident8
                if l > 0:
                    # ---- AllGather the fp8 t' table (bounce written during
                    # the previous layer's tile loop). The collective's APs
                    # are bitcast to bf16 so the NEFF sees a plain bf16
                    # AllGather of half the bytes.
                    table = dramp.tile([TBL, D], FP8,
                                       tag="table", addr_space="Shared")
                    nc.gpsimd.collective_compute(
                        "AllGather", mybir.AluOpType.bypass,
                        replica_groups=[list(range(NCORES))],
                        ins=[bounce.opt().bitcast(mybir.dt.bfloat16)],
                        outs=[table.opt().bitcast(mybir.dt.bfloat16)])
                if l < L - 1:
                    bounce = dramp.tile([PPC, D], FP8, tag="bounce")
                own_cur = own_t[l]
                own_nxt = own_t[l + 1] if l < L - 1 else None

                # ---- gather + scatter + relu (+ next-layer GEMM / JK) per tile
                new_hT = {}
                for h in range(2):
                    nht = hTp.tile([128, PPC], mybir.dt.bfloat16,
                                   tag=f"hT_{l}_{h}")
                    new_hT[h] = nht
                off = 0
                for b in range(NT):
                    sb_ = S[b]
                    agg = psA.tile([128, D], mybir.dt.float32, tag="agg")
                    # gather in balanced pieces of <=40 slots: caps the msg
                    # pool slot size without tiny tail pieces, and lets the
                    # scatter matmuls of piece k overlap gather k+1. The
                    # last tile before an AllGather is force-split so its
                    # scatter overlaps the second piece, shrinking the
                    # pre-AG tail.
                    np_ = -(-sb_ // 40)
                    if b == NT - 1:
                        np_ = max(np_, 2)
                    cuts = [sb_ * i // np_ for i in range(np_ + 1)]
                    pieces = list(zip(cuts[:-1], cuts[1:]))
                    for pi, (p0, p1) in enumerate(pieces):
                        msg = msgp.tile([128, p1 - p0, D], tdt,
                                        tag=f"msg{min(l, 1)}")
                        nc.gpsimd.dma_gather(
                            msg[:], table[:],
                            idx_sb[:, (off + p0 * 128) // 16:
                                   (off + p1 * 128) // 16],
                            num_idxs=(p1 - p0) * 128,
                            num_idxs_reg=(p1 - p0) * 128, elem_size=D,
                            single_packet=False)
                        for s in range(p1 - p0):
                            nc.tensor.matmul(agg[:], idt[:], msg[:, s, :],
                                             start=(pi == 0 and s == 0),
                                             stop=False)
                    off += sb_ * 128
                    # self-loop term from the SBUF stash (not gathered)
                    nc.tensor.matmul(agg[:], idt[:], own_cur[:, b, :],
                                     start=False, stop=True)
                    # h~ = relu(dinv^2 * agg)  (fold)  /  relu(dinv*agg + c)
                    hb = workp.tile([128, D], mybir.dt.bfloat16, tag="hb")
                    if fold:
                        nc.vector.tensor_scalar(
                            hb[:], agg[:], dinv_sb[:, b:b + 1], 0.0,
                            mybir.AluOpType.mult, mybir.AluOpType.max)
                    else:
                        tmp = workp.tile([128, D], mybir.dt.float32, tag="tmp")
                        nc.vector.tensor_scalar(
                            tmp[:], agg[:], dinv_sb[:, b:b + 1], None,
                            mybir.AluOpType.mult)
                        nc.vector.tensor_tensor(
                            tmp[:], tmp[:], c_sb[:, l, :], mybir.AluOpType.add)
                        nc.vector.tensor_scalar(
                            hb[:], tmp[:], 0.0, None, mybir.AluOpType.max)
                    # transpose h tile -> hT
                    for h in range(2):
                        tps = psT.tile([128, 128], mybir.dt.float32, tag="tr")
                        nc.tensor.matmul(tps[:], hb[:, 128 * h:128 * (h + 1)],
                                         ident[:], start=True, stop=True)
                        nc.scalar.activation(
                            new_hT[h][:, 128 * b:128 * (b + 1)], tps[:],
                            mybir.ActivationFunctionType.Copy)

                    if l < L - 1:
                        # next layer's local GEMM tile -> own stash -> bounce
                        tp = psB.tile([128, D], mybir.dt.float32, tag="gemm")
                        for h in range(2):
                            nc.tensor.matmul(
                                tp[:], new_hT[h][:, 128 * b:128 * (b + 1)],
                                W_sb[:, 2 * (l + 1) + h, :],
                                start=(h == 0), stop=(h == 1))
                        if fold:
                            nc.scalar.activation(
                                own_nxt[:, b, :], tp[:],
                                mybir.ActivationFunctionType.Copy)
                        else:
                            nc.vector.tensor_scalar(
                                own_nxt[:, b, :], tp[:], dinv_sb[:, b:b + 1],
                                None, mybir.AluOpType.mult)
                        nc.sync.dma_start(bounce[128 * b:128 * (b + 1), :],
                                          own_nxt[:, b, :])
                    else:
                        # JK final linear for this dst tile
                        op = psB.tile([128, D], mybir.dt.float32, tag="gemm")
                        k = 0
                        for l2 in range(L):
                            hTl = new_hT if l2 == l else hT_layers[l2]
                            for h in range(2):
                                nc.tensor.matmul(
                                    op[:], hTl[h][:, 128 * b:128 * (b + 1)],
                                    linw_sb[:, 2 * l2 + h, :],
                                    start=(k == 0), stop=(k == 5))
                                k += 1
                        ob = workp.tile([128, D], mybir.dt.float32, tag="ob")
                        if fold:
                            nc.vector.tensor_scalar(
                                ob[:], op[:], recip_sb[:, b:b + 1],
                                None, mybir.AluOpType.mult)
                        else:
                            nc.vector.tensor_copy(ob[:], op[:])
                        if not b_zero:
                            nc.vector.tensor_tensor(
                                ob[:], ob[:], linb_sb[:], mybir.AluOpType.add)
                        nc.sync.dma_start(out_d[128 * b:128 * (b + 1), :],
                                          ob[:])
                hT = new_hT
                hT_layers.append(new_hT)
    nc.compile()
    return nc


# ------------------------------------------------------------------- runner
def _in_maps(pre):
    return [
        {"xT": np.ascontiguousarray(pre["xT_pc"][cc]),
         "xTf": pre["xT_full"],
         "idx": np.ascontiguousarray(pre["idx_in"][cc]),
         "dinv": np.ascontiguousarray(pre["dinv_t"][cc]),
         "recip": np.ascontiguousarray(pre["recip_t"][cc])}
        for cc in range(NCORES)
    ]


def _unshard(pre, results):
    order = pre["order"]
    out = np.empty((N, D), np.float32)
    for cc in range(NCORES):
        nodes = order[cc::NCORES]
        out[nodes] = results[cc]["out"][:len(nodes)]
    return out


def kernel(x, edge_index, Ws, bs, bn_scale, bn_bias, bn_mean, bn_var,
           lin_w, lin_b):
    pre = _preprocess(x, edge_index, Ws, bs, bn_scale, bn_bias, bn_mean,
                      bn_var, lin_w, lin_b)
    nc = _build(pre["S"], pre["Wp"], pre["c"], pre["lin_w"], pre["lin_b"],
                pre["fold"])
    kw = {}
    if _TRACE:
        kw = dict(trace=True)
    res = run_bass_kernel_spmd(nc, _in_maps(pre), core_ids=list(range(NCORES)),
                               **kw)
    kernel.last_results = res
    return _unshard(pre, [res.results[cc] for cc in range(NCORES)])



# revision 44
# speedup vs baseline: 1.0089x; 1.0080x over previous
"""GCN (3-layer, JK-concat) Trainium2 kernel, 8-core SPMD.

Strategy (graph/data parallel, dst-sharded):
 - Nodes sorted by degree (desc), dealt round-robin to 8 cores -> each core
   owns 1250 dst nodes (padded to 1280 = 10 tiles of 128 partitions).
 - Symmetric norm folding: x is pre-scaled by dinv on host. When the BN/bias
   constant c == 0 (the shipped regime), the per-layer activation is kept in
   "scaled" form h~ = dinv*h = relu(dinv^2 * agg), which makes the next
   layer's table t'_{l+1} = h~ @ W'_{l+1} directly (no per-row scale), with a
   single 1/dinv row-scale applied once at the JK output.
 - Layer 1 needs NO collective: x is replicated input (xTf), so every core
   computes the ENTIRE t1' table locally (80 stripes of GEMM written to a
   local DRAM table). This removes AllGather #1 from the critical path.
 - Layers 2/3: t' tables are kept in fp8(e4m3), halving the AllGather
   bytes (~81us each in the CoreSim cost model vs ~147us for bf16). The
   collective's in/out APs are bitcast to bf16 containers so the NEFF sees
   an ordinary bf16 AllGather — declaring the collective tensors as fp8
   hard-crashes NRT exec on silicon (prior session), the bitcast does not.
 - Aggregation: per dst-tile dma_gather in balanced <=40-slot pieces
   (slot-major, degree-padded) -> identity-matmul scatter-accumulate into
   PSUM (fp32; fp8 identity/messages for layers 2/3) -> relu-scale (DVE)
   -> transpose (PE+ACT) for the next GEMM's lhsT. Self-loop messages are
   NOT gathered: each core keeps its own t' tiles in SBUF (own_t, one per
   layer with the layer's table dtype) and injects them with one extra
   accumulate-matmul per tile. The next layer's GEMM, bounce write, and
   the final JK linear are folded into the per-tile loop so they overlap
   the gather stream.
 - JK: out = (sum_l h~T_l.T @ lin_w_l) * (1/dinv) (+ lin_b), fp32 out.
"""
import sys
sys.path.insert(0, "/opt/trn_rl_repo")
import numpy as np
import ml_dtypes

import concourse.bass as bass
import concourse.bacc as bacc
import concourse.mybir as mybir
import concourse.tile as tile
from concourse.bass_utils import run_bass_kernel_spmd

N = 10000
E = 320000
D = 256
L = 3
BN_EPS = 1e-5
NCORES = 8
PPC = 1280           # padded nodes per core
NT = PPC // 128      # dst tiles per core
TBL = NCORES * PPC   # gather table rows
ZROW = 1279          # an always-zero table row (core 0 pad region)
BF16 = ml_dtypes.bfloat16

_TRACE = False       # set by test harness for profiling runs


# ----------------------------------------------------------------- host prep
def _preprocess(x, edge_index, Ws, bs, bn_scale, bn_bias, bn_mean, bn_var,
                lin_w, lin_b):
    src = np.asarray(edge_index[0], np.int64)
    dst = np.asarray(edge_index[1], np.int64)
    loops = np.arange(N, dtype=np.int64)

    # degree INCLUDES the self-loop (reference semantics) ...
    deg = (np.bincount(dst, minlength=N) + 1).astype(np.float64)
    dinv = np.where(deg > 0, 1.0 / np.sqrt(deg), 0.0).astype(np.float32)

    # deg-sorted round-robin node placement
    order = np.argsort(-deg, kind="stable")          # node of rank r
    core_of_rank = np.arange(N) % NCORES
    local_of_rank = np.arange(N) // NCORES
    core = np.empty(N, np.int64); local = np.empty(N, np.int64)
    core[order] = core_of_rank
    local[order] = local_of_rank
    trow = core * PPC + local                        # table row per node

    # ... but the gather schedule EXCLUDES self-loops: the own-t' term is
    # injected on-device from SBUF, saving one slot in every bucket.
    e_core = core[dst]
    e_tile = local[dst] // 128
    e_part = local[dst] % 128
    e_src_row = trow[src]

    # slot index of each edge within its (core, tile, part) bucket
    key = (e_core * NT + e_tile) * 128 + e_part
    order_e = np.argsort(key, kind="stable")
    key_s = key[order_e]
    starts = np.searchsorted(key_s, np.arange(NCORES * NT * 128))
    counts = np.diff(np.append(starts, len(key_s)))
    slot_in_bucket = np.arange(len(key_s)) - starts[key_s]

    counts3 = counts.reshape(NCORES, NT, 128)
    S = counts3.max(axis=(0, 2))                     # common slots per tile
    S = np.maximum(S, 1).astype(np.int64)

    # index arrays: per core, per tile: [S_b * 128] slot-major
    idx_lists = np.full((NCORES, int(S.sum()) * 128), ZROW, np.int32)
    tile_off = np.concatenate([[0], np.cumsum(S)])   # in slots
    ec = e_core[order_e]; et = e_tile[order_e]; ep = e_part[order_e]
    sl = slot_in_bucket
    flat_pos = (tile_off[et] + sl) * 128 + ep
    idx_lists[ec, flat_pos] = e_src_row[order_e]

    # wrap for dma_gather: position i -> partition i%16, col i//16; replicate x8
    tot = idx_lists.shape[1]
    assert tot % 16 == 0
    wrapped = idx_lists.reshape(NCORES, tot // 16, 16).transpose(0, 2, 1)
    idx_in = np.ascontiguousarray(
        np.tile(wrapped, (1, 8, 1))).astype(np.int16)  # [NCORES,128,tot//16]

    # BN folding
    rs = 1.0 / np.sqrt(np.asarray(bn_var, np.float64) + BN_EPS)
    colscale = rs * np.asarray(bn_scale, np.float64)           # [L,D]
    Wp = np.asarray(Ws, np.float64) * colscale[:, None, :]     # [L,D,D]
    c = ((np.asarray(bs, np.float64) - np.asarray(bn_mean, np.float64))
         * colscale + np.asarray(bn_bias, np.float64))         # [L,D]
    fold = bool(np.all(np.abs(c) < 1e-12))

    # per-core dinv-prescaled x^T
    xs = np.asarray(x, np.float32) * dinv[:, None]
    xT_pc = np.zeros((NCORES, D, PPC), np.float32)
    for cc in range(NCORES):
        nodes = order[cc::NCORES]
        xT_pc[cc, :, :len(nodes)] = xs[nodes].T
    # full table-ordered x^T (same on every core): lets each core compute the
    # ENTIRE layer-1 t' table locally, replacing the first AllGather.
    xT_full = np.zeros((D, TBL), np.float32)
    for cc in range(NCORES):
        nodes = order[cc::NCORES]
        xT_full[:, cc * PPC:cc * PPC + len(nodes)] = xs[nodes].T

    # per-core dst-tile dinv vectors: dinv^2 (fold) or dinv (generic), plus
    # 1/dinv for the JK output (fold only; 0 at pads).
    dinv_t = np.zeros((NCORES, 128, NT), np.float32)
    recip_t = np.zeros((NCORES, 128, NT), np.float32)
    for cc in range(NCORES):
        nodes = order[cc::NCORES]                    # nodes of core cc by local
        dv = np.zeros(PPC, np.float32)
        dv[:len(nodes)] = dinv[nodes]
        dvt = dv.reshape(NT, 128).T
        dinv_t[cc] = dvt * dvt if fold else dvt
        with np.errstate(divide="ignore"):
            recip_t[cc] = np.where(dvt > 0, 1.0 / dvt, 0.0)
    return dict(
        S=S, idx_in=idx_in, xT_pc=xT_pc.astype(BF16),
        xT_full=np.ascontiguousarray(xT_full.astype(BF16)), dinv_t=dinv_t,
        recip_t=recip_t, fold=fold,
        Wp=Wp.astype(np.float32), c=c.astype(np.float32),
        lin_w=np.asarray(lin_w, np.float32), lin_b=np.asarray(lin_b, np.float32),
        order=order,
    )


# -------------------------------------------------------------- device build
def _build(S, Wp, c, lin_w, lin_b, fold):
    S = [int(s) for s in S]
    tot = sum(S) * 128
    b_zero = bool(np.all(np.abs(lin_b) < 1e-12))

    nc = bacc.Bacc("TRN2", target_bir_lowering=False, debug=False,
                   enable_asserts=True, num_devices=NCORES)
    xT_d = nc.dram_tensor("xT", [D, PPC], mybir.dt.bfloat16,
                          kind="ExternalInput")
    xTf_d = nc.dram_tensor("xTf", [D, TBL], mybir.dt.bfloat16,
                           kind="ExternalInput")
    idx_d = nc.dram_tensor("idx", [128, tot // 16], mybir.dt.int16,
                           kind="ExternalInput")
    dinv_d = nc.dram_tensor("dinv", [128, NT], mybir.dt.float32,
                            kind="ExternalInput")
    recip_d = nc.dram_tensor("recip", [128, NT], mybir.dt.float32,
                             kind="ExternalInput")
    out_d = nc.dram_tensor("out", [PPC, D], mybir.dt.float32,
                           kind="ExternalOutput")

    ident_d = nc.inline_tensor(np.eye(128, dtype=BF16), name="ident")
    ident8_d = nc.inline_tensor(
        np.eye(128).astype(ml_dtypes.float8_e4m3), name="ident8")
    Wp_d = nc.inline_tensor(Wp.astype(BF16), name="Wp")            # [L,D,D]
    linw_d = nc.inline_tensor(lin_w.astype(BF16), name="linw")     # [768,D]
    if not fold:
        c_d = nc.inline_tensor(
            np.broadcast_to(c[:, None, :], (L, 128, D)).copy(), name="cvec")
    if not b_zero:
        linb_d = nc.inline_tensor(
            np.broadcast_to(lin_b[None, :], (128, D)).copy(), name="linb")

    with tile.TileContext(nc) as tc:
        with (
            tc.tile_pool(name="const", bufs=1) as constp,
            tc.tile_pool(name="hT", bufs=1) as hTp,
            tc.tile_pool(name="msg", bufs=2) as msgp,
            tc.tile_pool(name="work", bufs=3) as workp,
            tc.tile_pool(name="dram", bufs=2, space="DRAM") as dramp,
            tc.tile_pool(name="psA", bufs=2, space="PSUM") as psA,
            tc.tile_pool(name="psB", bufs=2, space="PSUM") as psB,
            tc.tile_pool(name="psT", bufs=2, space="PSUM") as psT,
        ):
            # ---- critical-path constants first: W (for the L1 GEMM), x^T.
            W_sb = constp.tile([128, L * 2, D], mybir.dt.bfloat16, tag="W")
            nc.sync.dma_start(
                W_sb[:], Wp_d.ap().rearrange("l (h p) d -> p (l h) d", p=128))
            xT_sb = constp.tile([128, 2, PPC], mybir.dt.bfloat16, tag="xTpc")
            for h in range(2):
                nc.sync.dma_start(xT_sb[:, h, :],
                                  xT_d[128 * h:128 * (h + 1), :])
            # full table-ordered x^T: every core computes the ENTIRE layer-1
            # t' table locally (x is replicated input), replacing AG #1.
            # Two column-half tiles so the GEMM stream starts after the
            # first half's load instead of the full 5.2MB.
            HTB = TBL // 2
            xTfs = []
            for half in range(2):
                t = constp.tile([128, 2, HTB], mybir.dt.bfloat16,
                                tag=f"xTf{half}")
                for h in range(2):
                    nc.sync.dma_start(
                        t[:, h, :],
                        xTf_d[128 * h:128 * (h + 1),
                              half * HTB:(half + 1) * HTB])
                xTfs.append(t)

            # own-t' SBUF stash, one per layer (L1 bf16; L2/L3 fp8 to halve
            # the AllGather bytes). Feeds the self-loop injection.
            FP8 = mybir.dt.float8e4
            own_t = [hTp.tile([128, NT, D],
                              mybir.dt.bfloat16 if i == 0 else FP8,
                              tag=f"own_{i}", name=f"own_t{i}")
                     for i in range(L)]

            # ---- layer-1: replicated GEMM over all 80 table stripes,
            # written straight to the local DRAM table (no collective).
            # Own tiles additionally land in own_t[0] for self-injection.
            for b in range(NT):
                tp = psB.tile([128, D], mybir.dt.float32, tag="gemm")
                for h in range(2):
                    nc.tensor.matmul(
                        tp[:], xT_sb[:, h, 128 * b:128 * (b + 1)],
                        W_sb[:, h, :], start=(h == 0), stop=(h == 1))
                if b % 2 == 0:
                    nc.scalar.activation(
                        own_t[0][:, b, :], tp[:],
                        mybir.ActivationFunctionType.Copy)
                else:
                    nc.vector.tensor_copy(own_t[0][:, b, :], tp[:])
            table1 = dramp.tile([TBL, D], mybir.dt.bfloat16, tag="table1")
            G1 = 8                      # stripes per staged table write
            for g in range(TBL // 128 // G1):
                stage = workp.tile([128, G1, D], mybir.dt.bfloat16,
                                   tag="l1stage")
                for j in range(G1):
                    s = g * G1 + j
                    src = xTfs[s // (HTB // 128)]
                    so = s % (HTB // 128)
                    tp = psB.tile([128, D], mybir.dt.float32, tag="gemm")
                    for h in range(2):
                        nc.tensor.matmul(
                            tp[:], src[:, h, 128 * so:128 * (so + 1)],
                            W_sb[:, h, :], start=(h == 0), stop=(h == 1))
                    if j % 2 == 0:
                        nc.scalar.activation(
                            stage[:, j, :], tp[:],
                            mybir.ActivationFunctionType.Copy)
                    else:
                        nc.vector.tensor_copy(stage[:, j, :], tp[:])
                nc.sync.dma_start(
                    table1[128 * G1 * g:128 * G1 * (g + 1), :]
                    .rearrange("(k p) d -> p k d", p=128),
                    stage[:])

            # ---- remaining constants (overlap the collective)
            ident = constp.tile([128, 128], mybir.dt.bfloat16, tag="ident")
            nc.sync.dma_start(ident[:], ident_d[:])
            ident8 = constp.tile([128, 128], FP8, tag="ident8")
            nc.sync.dma_start(ident8[:], ident8_d[:])


            idx_sb = constp.tile([128, tot // 16], mybir.dt.int16, tag="idx")
            nc.sync.dma_start(idx_sb[:], idx_d[:])
            dinv_sb = constp.tile([128, NT], mybir.dt.float32, tag="dinv")
            nc.sync.dma_start(dinv_sb[:], dinv_d[:])
            if fold:
                recip_sb = constp.tile([128, NT], mybir.dt.float32, tag="recip")
                nc.sync.dma_start(recip_sb[:], recip_d[:])
            linw_sb = constp.tile([128, L * 2, D], mybir.dt.bfloat16,
                                  tag="linw")
            nc.sync.dma_start(
                linw_sb[:], linw_d.ap().rearrange("(k p) d -> p k d", p=128))
            if not fold:
                c_sb = constp.tile([128, L, D], mybir.dt.float32, tag="cvec")
                nc.sync.dma_start(c_sb[:], c_d.ap().rearrange("l p d -> p l d"))
            if not b_zero:
                linb_sb = constp.tile([128, D], mybir.dt.float32, tag="linb")
                nc.sync.dma_start(linb_sb[:], linb_d.ap())

            hT = None
            hT_layers = []
            bounce = None
            table = table1
            for l in range(L):
                tdt = mybir.dt.bfloat16 if l == 0 else FP8
                idt = ident if l == 0 else ident8
                if l > 0:
                    # ---- AllGather the fp8 t' table (bounce written during
                    # the previous layer's tile loop). The collective's APs
                    # are bitcast to bf16 so the NEFF sees a plain bf16
                    # AllGather of half the bytes.
                    table = dramp.tile([TBL, D], FP8,
                                       tag="table", addr_space="Shared")
                    nc.gpsimd.collective_compute(
                        "AllGather", mybir.AluOpType.bypass,
                        replica_groups=[list(range(NCORES))],
                        ins=[bounce.opt().bitcast(mybir.dt.bfloat16)],
                        outs=[table.opt().bitcast(mybir.dt.bfloat16)])
                if l < L - 1:
                    bounce = dramp.tile([PPC, D], FP8, tag="bounce")
                own_cur = own_t[l]
                own_nxt = own_t[l + 1] if l < L - 1 else None

                # ---- gather + scatter + relu (+ next-layer GEMM / JK) per tile
                new_hT = {}
                for h in range(2):
                    nht = hTp.tile([128, PPC], mybir.dt.bfloat16,
                                   tag=f"hT_{l}_{h}")
                    new_hT[h] = nht
                off = 0
                for b in range(NT):
                    sb_ = S[b]
                    agg = psA.tile([128, D], mybir.dt.float32, tag="agg")
                    # gather in balanced pieces of <=40 slots: caps the msg
                    # pool slot size without tiny tail pieces, and lets the
                    # scatter matmuls of piece k overlap gather k+1. The
                    # last tile before an AllGather is force-split so its
                    # scatter overlaps the second piece, shrinking the
                    # pre-AG tail.
                    np_ = -(-sb_ // 40)
                    if b == NT - 1:
                        np_ = max(np_, 2)
                    cuts = [sb_ * i // np_ for i in range(np_ + 1)]
                    pieces = list(zip(cuts[:-1], cuts[1:]))
                    for pi, (p0, p1) in enumerate(pieces):
                        msg = msgp.tile([128, p1 - p0, D], tdt,
                                        tag=f"msg{min(l, 1)}")
                        nc.gpsimd.dma_gather(
                            msg[:], table[:],
                            idx_sb[:, (off + p0 * 128) // 16:
                                   (off + p1 * 128) // 16],
                            num_idxs=(p1 - p0) * 128,
                            num_idxs_reg=(p1 - p0) * 128, elem_size=D,
                            single_packet=False)
                        for s in range(p1 - p0):
                            nc.tensor.matmul(agg[:], idt[:], msg[:, s, :],
                                             start=(pi == 0 and s == 0),
                                             stop=False)
                    off += sb_ * 128
                    # self-loop term from the SBUF stash (not gathered)
                    nc.tensor.matmul(agg[:], idt[:], own_cur[:, b, :],
                                     start=False, stop=True)
                    # h~ = relu(dinv^2 * agg)  (fold)  /  relu(dinv*agg + c)
                    hb = workp.tile([128, D], mybir.dt.bfloat16, tag="hb")
                    if fold:
                        nc.vector.tensor_scalar(
                            hb[:], agg[:], dinv_sb[:, b:b + 1], 0.0,
                            mybir.AluOpType.mult, mybir.AluOpType.max)
                    else:
                        tmp = workp.tile([128, D], mybir.dt.float32, tag="tmp")
                        nc.vector.tensor_scalar(
                            tmp[:], agg[:], dinv_sb[:, b:b + 1], None,
                            mybir.AluOpType.mult)
                        nc.vector.tensor_tensor(
                            tmp[:], tmp[:], c_sb[:, l, :], mybir.AluOpType.add)
                        nc.vector.tensor_scalar(
                            hb[:], tmp[:], 0.0, None, mybir.AluOpType.max)
                    # transpose h tile -> hT
                    for h in range(2):
                        tps = psT.tile([128, 128], mybir.dt.float32, tag="tr")
                        nc.tensor.matmul(tps[:], hb[:, 128 * h:128 * (h + 1)],
                                         ident[:], start=True, stop=True)
                        nc.scalar.activation(
                            new_hT[h][:, 128 * b:128 * (b + 1)], tps[:],
                            mybir.ActivationFunctionType.Copy)

                    if l < L - 1:
                        # next layer's local GEMM tile -> own stash -> bounce
                        tp = psB.tile([128, D], mybir.dt.float32, tag="gemm")
                        for h in range(2):
                            nc.tensor.matmul(
                                tp[:], new_hT[h][:, 128 * b:128 * (b + 1)],
                                W_sb[:, 2 * (l + 1) + h, :],
                                start=(h == 0), stop=(h == 1))
                        if fold:
                            nc.scalar.activation(
                                own_nxt[:, b, :], tp[:],
                                mybir.ActivationFunctionType.Copy)
                        else:
                            nc.vector.tensor_scalar(
                                own_nxt[:, b, :], tp[:], dinv_sb[:, b:b + 1],
                                None, mybir.AluOpType.mult)
                        nc.sync.dma_start(bounce[128 * b:128 * (b + 1), :],
                                          own_nxt[:, b, :])
                    else:
                        # JK final linear for this dst tile
                        op = psB.tile([128, D], mybir.dt.float32, tag="gemm")
                        k = 0
                        for l2 in range(L):
                            hTl = new_hT if l2 == l else hT_layers[l2]
                            for h in range(2):
                                nc.tensor.matmul(
                                    op[:], hTl[h][:, 128 * b:128 * (b + 1)],
                                    linw_sb[:, 2 * l2 + h, :],
                                    start=(k == 0), stop=(k == 5))
                                k += 1
                        ob = workp.tile([128, D], mybir.dt.float32, tag="ob")
                        if fold:
                            nc.vector.tensor_scalar(
                                ob[:], op[:], recip_sb[:, b:b + 1],
                                None, mybir.AluOpType.mult)
                        else:
                            nc.vector.tensor_copy(ob[:], op[:])
                        if not b_zero:
                            nc.vector.tensor_tensor(
                                ob[:], ob[:], linb_sb[:], mybir.AluOpType.add)
                        nc.sync.dma_start(out_d[128 * b:128 * (b + 1), :],
                                          ob[:])
                hT = new_hT
                hT_layers.append(new_hT)
    nc.compile()
    return nc


# ------------------------------------------------------------------- runner
def _in_maps(pre):
    return [
        {"xT": np.ascontiguousarray(pre["xT_pc"][cc]),
         "xTf": pre["xT_full"],
         "idx": np.ascontiguousarray(pre["idx_in"][cc]),
         "dinv": np.ascontiguousarray(pre["dinv_t"][cc]),
         "recip": np.ascontiguousarray(pre["recip_t"][cc])}
        for cc in range(NCORES)
    ]


def _unshard(pre, results):
    order = pre["order"]
    out = np.empty((N, D), np.float32)
    for cc in range(NCORES):
        nodes = order[cc::NCORES]
        out[nodes] = results[cc]["out"][:len(nodes)]
    return out


def kernel(x, edge_index, Ws, bs, bn_scale, bn_bias, bn_mean, bn_var,
           lin_w, lin_b):
    pre = _preprocess(x, edge_index, Ws, bs, bn_scale, bn_bias, bn_mean,
                      bn_var, lin_w, lin_b)
    nc = _build(pre["S"], pre["Wp"], pre["c"], pre["lin_w"], pre["lin_b"],
                pre["fold"])
    kw = {}
    if _TRACE:
        kw = dict(trace=True)
    res = run_bass_kernel_spmd(nc, _in_maps(pre), core_ids=list(range(NCORES)),
                               **kw)
    kernel.last_results = res
    return _unshard(pre, [res.results[cc] for cc in range(NCORES)])

